# revision 9
# baseline (speedup 1.0000x reference)
"""Trainium2 Bass kernel for batched windowed multi-head attention.

Shapes: x (8, 64, 256, 512) f32, H=8 heads, D=64.
Sharding: data-parallel over batch dim B=8 -> 1 batch row per NeuronCore.

v2 design (vs baseline):
- x cast to bf16 on host; loaded pre-transposed via the XBAR DMA-transpose
  (no PE transposes, no PSUM->SBUF copies for xT).
- q/k/v projections in bf16 (same PE rate as fp32r at N>=256, half the
  SBUF/DMA traffic); q/k bias folded into the PSUM->SBUF cast on ACT.
- mask+pos_bias folded multiplicatively: host precomputes
  emp = exp(mask^T + pos^T) in bf16, streamed per window over DMA; on-chip
  softmax numerator is exp(scores) * emp via one DVE bf16 multiply per head
  (replaces the Pool add + DVE add chains of the baseline).
- denominators via a 64-wide ones BLOCK appended to V: za = [v|1]^T @ exp
  gives rows 64..127 all equal to the softmax denominator, so the
  reciprocal + normalize are two plain DVE ops, no broadcasts needed.
- v/out biases folded into the projection matmuls via a K=1 ones-row
  matmul (PE) instead of DVE scalar_tensor_tensor ops.
"""
import os
import numpy as np
import ml_dtypes

import concourse.bass as bass
import concourse.mybir as mybir
import concourse.tile as tile
from concourse import bacc
from concourse.bass_utils import run_bass_kernel_spmd

B, W, S, E = 8, 64, 256, 512
H, D = 8, 64
SCALE = D ** -0.5
NCORES = 8
F32 = mybir.dt.float32
F32R = mybir.dt.float32r
BF16 = mybir.dt.bfloat16
AOp = mybir.AluOpType
AF = mybir.ActivationFunctionType
BFNP = ml_dtypes.bfloat16


def _emit(nc, tc, ctx, n_w, d):
    """Emit the per-core program: n_w windows of MHA."""
    const = ctx.enter_context(tc.tile_pool(name="const", bufs=1))

    # --- one-time: weights, biases ---
    w_sb = {}
    for name in ("wq", "wk", "wv"):
        t = const.tile([128, 4, E], BF16, tag=name)
        nc.sync.dma_start(t[:], d[name][:])
        w_sb[name] = t
    with tc.tile_pool(name="wstage", bufs=1) as wstage:
        st = wstage.tile([128, 4, E], F32, tag="wst")
        nc.sync.dma_start(st[:], d["wp"][:])
        wp_sb = const.tile([128, 4, E], F32R, tag="wp")
        nc.vector.tensor_copy(wp_sb[:], st[:])
        bst = wstage.tile([1, E], F32, tag="bst")
        nc.sync.dma_start(bst[:], d["bp_row"][:])
        bp_sb = const.tile([1, E], F32R, tag="bp")
        nc.vector.tensor_copy(bp_sb[:], bst[:])
        ost = wstage.tile([1, 128], F32, tag="ost")
        nc.sync.dma_start(ost[:], d["ones_f"][:])
        ones_r = const.tile([1, 128], F32R, tag="ones_r")
        nc.vector.tensor_copy(ones_r[:], ost[:])

    bq_sb = const.tile([128, 4], F32)
    nc.sync.dma_start(bq_sb[:], d["bq"][:])
    bk_sb = const.tile([128, 4], F32)
    nc.sync.dma_start(bk_sb[:], d["bk"][:])
    bv_sb = const.tile([1, E], BF16)
    nc.sync.dma_start(bv_sb[:], d["bv_row"][:])

    ones_b = const.tile([1, 128], BF16)
    nc.sync.dma_start(ones_b[:], d["ones_b"][:])
    vones = const.tile([128, 2, H, 64], BF16)
    nc.sync.dma_start(vones[:], d["vones"][:])

    # --- pools for the per-window-pair pipeline ---
    emp_p = ctx.enter_context(tc.tile_pool(name="emp", bufs=4))
    xt_p = ctx.enter_context(tc.tile_pool(name="xt", bufs=2))
    qkv_p = ctx.enter_context(tc.tile_pool(name="qkv", bufs=2))
    et_p = ctx.enter_context(tc.tile_pool(name="et", bufs=2))
    expt_p = ctx.enter_context(tc.tile_pool(name="expt", bufs=3))
    rec_p = ctx.enter_context(tc.tile_pool(name="rec", bufs=4))
    zt_p = ctx.enter_context(tc.tile_pool(name="zt", bufs=2))
    outs_p = ctx.enter_context(tc.tile_pool(name="outs", bufs=2))

    ps_pj = ctx.enter_context(tc.tile_pool(name="ps_pj", bufs=3, space="PSUM"))
    ps_sc = ctx.enter_context(tc.tile_pool(name="ps_sc", bufs=2, space="PSUM"))
    ps_z = ctx.enter_context(tc.tile_pool(name="ps_z", bufs=3, space="PSUM"))

    def phase_a(p):
        """Load window pair (2p, 2p+1); project q/k/v (dense PE work).
        q/k matmuls cover both windows at once (N=512)."""
        xT2 = xt_p.tile([128, 2, 4, S], BF16, tag="xT", name=f"xT{p}")
        emps = []
        for w2 in range(2):
            nc.sync.dma_start_transpose(xT2[:, w2], d["x"][2 * p + w2])
            emp = emp_p.tile([128, H, 2, S], BF16, tag="emp", name=f"emp{p}_{w2}")
            nc.sync.dma_start(emp[:], d["emp"][2 * p + w2])
            emps.append(emp)

        qT2 = qkv_p.tile([128, 2, 4, S], F32R, tag="qT", name=f"qT{p}")
        kT2 = qkv_p.tile([128, 2, 4, S], F32R, tag="kT", name=f"kT{p}")
        vAs = []
        for w2 in range(2):
            vA = qkv_p.tile([128, 2, H, 128], BF16, tag=f"vA{w2}",
                            name=f"vA{p}_{w2}")
            nc.gpsimd.tensor_copy(vA[:, :, :, 0:64], vones[:])
            vAs.append(vA)

        def qk_chunk(oc, wt, dst, bias):
            pp = ps_pj.tile([128, 2, S], F32, tag="pj", name=f"pp{p}_{wt}_{oc}")
            for ic in range(4):
                nc.tensor.matmul(pp[:], w_sb[wt][:, ic, oc * 128:(oc + 1) * 128],
                                 xT2[:, :, ic, :], start=(ic == 0), stop=(ic == 3))
            nc.scalar.activation(dst[:, :, oc, :], pp[:], AF.Identity,
                                 bias=bias[:, oc:oc + 1])

        def v_chunk(w2, sc):
            pv = ps_pj.tile([128, E], F32, tag="pj", name=f"pv{p}_{w2}_{sc}")
            for ic in range(4):
                nc.tensor.matmul(pv[:], xT2[:, w2, ic, sc * 128:(sc + 1) * 128],
                                 w_sb["wv"][:, ic], start=(ic == 0), stop=False)
            nc.tensor.matmul(pv[:], ones_b[:], bv_sb[:], start=False, stop=True)
            nc.scalar.copy(vAs[w2][:, sc, :, 64:128],
                           pv[:].rearrange("p (h o) -> p h o", h=H))

        chunks = []
        for oc in range(4):
            chunks.append(lambda oc=oc: qk_chunk(oc, "wq", qT2, bq_sb))
            chunks.append(lambda oc=oc: qk_chunk(oc, "wk", kT2, bk_sb))
        for w2 in range(2):
            chunks.append(lambda w2=w2: v_chunk(w2, 0))
            chunks.append(lambda w2=w2: v_chunk(w2, 1))
        return (qT2, kT2, vAs, emps), chunks

    def phase_b(p, qT2, kT2, vAs, emps):
        """Attention + output projection for window pair p."""
        zTs = [zt_p.tile([128, 4, S], F32R, tag=f"zT{w2}", name=f"zT{p}_{w2}")
               for w2 in range(2)]

        def head(w2, h):
            oc, prow = h // 2, (h % 2) * 64
            w = 2 * p + w2
            # transposed scores: [128 (j%128), jc, i]
            sT = ps_sc.tile([128, 2, S], F32, tag="sc", name=f"sT{w}_{h}")
            for jc in range(2):
                nc.tensor.matmul(sT[:, jc],
                                 kT2[prow:prow + 64, w2, oc, jc * 128:(jc + 1) * 128],
                                 qT2[prow:prow + 64, w2, oc], start=True, stop=True)
            et = et_p.tile([128, 2, S], BF16, tag="et", name=f"et{w}_{h}")
            nc.scalar.activation(et[:], sT[:], AF.Exp)
            expT = expt_p.tile([128, 2, S], BF16, tag="expT", name=f"expT{w}_{h}")
            nc.vector.tensor_tensor(expT[:], et[:], emps[w2][:, h], AOp.mult)
            # za rows 0-63 each = sum_j exp (den), rows 64-127 = v_h^T @ exp
            za = ps_z.tile([128, S], F32, tag="zz", name=f"za{w}_{h}")
            for jc in range(2):
                nc.tensor.matmul(za[:], vAs[w2][:, jc, h], expT[:, jc],
                                 start=(jc == 0), stop=(jc == 1))
            rec = rec_p.tile([64, S], F32, tag="rec", name=f"rec{w}_{h}")
            nc.vector.reciprocal_approx_fast(rec[:], za[0:64, :])
            nc.vector.tensor_tensor(zTs[w2][prow:prow + 64, oc], za[64:128, :],
                                    rec[:], AOp.mult)

        def tail(w2):
            w = 2 * p + w2
            out_sb = outs_p.tile([128, 2, E], F32, tag="osb", name=f"osb{w}")
            for sc in range(2):
                po = ps_pj.tile([128, E], F32, tag="pj", name=f"po{w}_{sc}")
                for ec in range(4):
                    nc.tensor.matmul(po[:], zTs[w2][:, ec, sc * 128:(sc + 1) * 128],
                                     wp_sb[:, ec], start=(ec == 0), stop=False)
                nc.tensor.matmul(po[:], ones_r[:], bp_sb[:], start=False, stop=True)
                if sc == 0:
                    nc.scalar.copy(out_sb[:, sc], po[:])
                else:
                    nc.vector.tensor_copy(out_sb[:, sc], po[:])
            nc.sync.dma_start(d["out"][w].rearrange("(c p) e -> p c e", p=128), out_sb[:])

        fns = []
        for w2 in range(2):
            fns.extend(lambda h=h, w2=w2: head(w2, h) for h in range(H))
            fns.append(lambda w2=w2: tail(w2))
        return fns

    # chunk i of the next pair is placed after head i+CHUNK_LAG of this pair,
    # so the pair's input DMAs have a few heads of compute as latency cover.
    CHUNK_LAG = 3
    n_p = n_w // 2
    prev = None
    for p in range(n_p):
        cur, chunks = phase_a(p)
        if prev is not None:
            bfns = phase_b(p - 1, *prev)
            seq = []
            ci = 0
            for i, fn in enumerate(bfns):
                seq.append(fn)
                if i >= CHUNK_LAG and ci < len(chunks):
                    seq.append(chunks[ci]); ci += 1
            seq.extend(chunks[ci:])
            for fn in seq:
                fn()
        else:
            for fn in chunks:
                fn()
        prev = cur
    for fn in phase_b(n_p - 1, *prev):
        fn()


def _decl(nc, n_w):
    return {
        "x": nc.dram_tensor("x", [n_w, S, E], BF16, kind="ExternalInput"),
        "emp": nc.dram_tensor("emp", [n_w, 128, H, 2, S], BF16, kind="ExternalInput"),
        "wq": nc.dram_tensor("wq", [128, 4, E], BF16, kind="ExternalInput"),
        "wk": nc.dram_tensor("wk", [128, 4, E], BF16, kind="ExternalInput"),
        "wv": nc.dram_tensor("wv", [128, 4, E], BF16, kind="ExternalInput"),
        "wp": nc.dram_tensor("wp", [128, 4, E], F32, kind="ExternalInput"),
        "bq": nc.dram_tensor("bq", [128, 4], F32, kind="ExternalInput"),
        "bk": nc.dram_tensor("bk", [128, 4], F32, kind="ExternalInput"),
        "bv_row": nc.dram_tensor("bv_row", [1, E], BF16, kind="ExternalInput"),
        "bp_row": nc.dram_tensor("bp_row", [1, E], F32, kind="ExternalInput"),
        "ones_b": nc.dram_tensor("ones_b", [1, 128], BF16, kind="ExternalInput"),
        "ones_f": nc.dram_tensor("ones_f", [1, 128], F32, kind="ExternalInput"),
        "vones": nc.dram_tensor("vones", [128, 2, H, 64], BF16, kind="ExternalInput"),
        "out": nc.dram_tensor("out", [n_w, S, E], F32, kind="ExternalOutput"),
    }


def _build(n_w):
    nc = bacc.Bacc("TRN2", target_bir_lowering=False, debug=False)
    d = _decl(nc, n_w)
    from contextlib import ExitStack
    with tile.TileContext(nc) as tc, ExitStack() as ctx:
        _emit(nc, tc, ctx, n_w, d)
    nc.compile()
    return nc


_NC_CACHE = {}


def _get_nc(n_w):
    if n_w not in _NC_CACHE:
        _NC_CACHE[n_w] = _build(n_w)
    return _NC_CACHE[n_w]


def _host_prep(mask, Wq, bq, Wk, bk, Wv, bv, Wp, bp, pos_bias, n_w):
    """Shared (replicated) input tensors, host-side layout prep."""
    f = np.float32

    def chunk_w(wt, dtype):  # [out,in] torch layout -> [128 (p), 4 (ic), out]
        wt_t = np.asarray(wt, f).T  # [in, out]
        return np.ascontiguousarray(
            wt_t.reshape(4, 128, E).transpose(1, 0, 2)).astype(dtype)

    wq_t = chunk_w(np.asarray(Wq, f) * SCALE, BFNP)
    wk_t = chunk_w(Wk, BFNP)
    wv_t = chunk_w(Wv, BFNP)
    wp_t = chunk_w(Wp, f)
    bq_t = np.ascontiguousarray((np.asarray(bq, f) * SCALE).reshape(4, 128).T)
    bk_t = np.ascontiguousarray(np.asarray(bk, f).reshape(4, 128).T)
    bv_row = np.asarray(bv, f).reshape(1, E).astype(BFNP)
    bp_row = np.ascontiguousarray(np.asarray(bp, f).reshape(1, E))
    # emp = exp(mask^T + pos^T), laid out [w, p (j%128), h, jc (j//128), i]
    maskT = np.asarray(mask, f)[0, :n_w, 0].transpose(0, 2, 1)  # [w, j, i]
    posT = np.asarray(pos_bias, f).transpose(0, 2, 1)           # [h, j, i]
    empf = np.exp(maskT[:, None] + posT[None])                  # [w, h, j, i]
    emp = np.ascontiguousarray(
        empf.reshape(n_w, H, 2, 128, S).transpose(0, 3, 1, 2, 4)).astype(BFNP)
    return {
        "wq": wq_t, "wk": wk_t, "wv": wv_t, "wp": wp_t,
        "bq": bq_t, "bk": bk_t, "bv_row": bv_row, "bp_row": bp_row,
        "emp": emp,
        "ones_b": np.ones((1, 128), BFNP),
        "ones_f": np.ones((1, 128), f),
        "vones": np.ones((128, 2, H, 64), BFNP),
    }


def kernel(x, mask, Wq, bq, Wk, bk, Wv, bv, Wp, bp, pos_bias, _trace=False):
    n_w = int(os.environ.get("KERNEL_NW", W))
    n_cores = NCORES
    x = np.asarray(x, np.float32)[:, :n_w].astype(BFNP)
    shared = _host_prep(mask, Wq, bq, Wk, bk, Wv, bv, Wp, bp, pos_bias, n_w)

    in_maps = []
    for c in range(n_cores):
        m = dict(shared)
        m["x"] = np.ascontiguousarray(x[c % B])
        in_maps.append(m)

    nc = _get_nc(n_w)
    res = run_bass_kernel_spmd(nc, in_maps, list(range(n_cores)), trace=_trace,
                               tmpdir=(os.environ.get("KERNEL_TRACE_DIR") if _trace else None))
    out = np.stack([res.results[c]["out"] for c in range(B)], axis=0)
    if _trace:
        kernel._last_exec_time_ns = res.exec_time_ns
        kernel._last_results = res
    return out


# revision 12
# speedup vs baseline: 1.0316x; 1.0316x over previous
"""Trainium2 Bass kernel for batched windowed multi-head attention.

Shapes: x (8, 64, 256, 512) f32, H=8 heads, D=64.
Sharding: data-parallel over batch dim B=8 -> 1 batch row per NeuronCore.

v2 design (vs baseline):
- x cast to bf16 on host; loaded pre-transposed via the XBAR DMA-transpose
  (no PE transposes, no PSUM->SBUF copies for xT).
- q/k/v projections in bf16 (same PE rate as fp32r at N>=256, half the
  SBUF/DMA traffic); q/k bias folded into the PSUM->SBUF cast on ACT.
- mask+pos_bias folded multiplicatively: host precomputes
  emp = exp(mask^T + pos^T) in bf16, streamed per window over DMA; on-chip
  softmax numerator is exp(scores) * emp via one DVE bf16 multiply per head
  (replaces the Pool add + DVE add chains of the baseline).
- denominators via a 64-wide ones BLOCK appended to V: za = [v|1]^T @ exp
  gives rows 64..127 all equal to the softmax denominator, so the
  reciprocal + normalize are two plain DVE ops, no broadcasts needed.
- v/out biases folded into the projection matmuls via a K=1 ones-row
  matmul (PE) instead of DVE scalar_tensor_tensor ops.
"""
import os
import numpy as np
import ml_dtypes

import concourse.bass as bass
import concourse.mybir as mybir
import concourse.tile as tile
from concourse import bacc
from concourse.bass_utils import run_bass_kernel_spmd

B, W, S, E = 8, 64, 256, 512
H, D = 8, 64
SCALE = D ** -0.5
NCORES = 8
F32 = mybir.dt.float32
F32R = mybir.dt.float32r
BF16 = mybir.dt.bfloat16
AOp = mybir.AluOpType
AF = mybir.ActivationFunctionType
BFNP = ml_dtypes.bfloat16


def _emit(nc, tc, ctx, n_w, d):
    """Emit the per-core program: n_w windows of MHA."""
    const = ctx.enter_context(tc.tile_pool(name="const", bufs=1))

    # --- one-time: weights, biases ---
    w_sb = {}
    for name in ("wq", "wk", "wv"):
        t = const.tile([128, 4, E], BF16, tag=name)
        nc.sync.dma_start(t[:], d[name][:])
        w_sb[name] = t
    wp_sb = const.tile([128, 4, E], BF16, tag="wp")
    nc.sync.dma_start(wp_sb[:], d["wp"][:])
    bp_sb = const.tile([1, E], BF16, tag="bp")
    nc.sync.dma_start(bp_sb[:], d["bp_row"][:])

    bq_sb = const.tile([128, 4], F32)
    nc.sync.dma_start(bq_sb[:], d["bq"][:])
    bk_sb = const.tile([128, 4], F32)
    nc.sync.dma_start(bk_sb[:], d["bk"][:])
    bv_sb = const.tile([1, E], BF16)
    nc.sync.dma_start(bv_sb[:], d["bv_row"][:])

    ones_b = const.tile([1, 128], BF16)
    nc.sync.dma_start(ones_b[:], d["ones_b"][:])
    vones = const.tile([128, 2, H, 64], BF16)
    nc.sync.dma_start(vones[:], d["vones"][:])

    # --- pools for the per-window-pair pipeline ---
    emp_p = ctx.enter_context(tc.tile_pool(name="emp", bufs=4))
    xt_p = ctx.enter_context(tc.tile_pool(name="xt", bufs=2))
    qkv_p = ctx.enter_context(tc.tile_pool(name="qkv", bufs=2))
    et_p = ctx.enter_context(tc.tile_pool(name="et", bufs=3))
    expt_p = ctx.enter_context(tc.tile_pool(name="expt", bufs=4))
    rec_p = ctx.enter_context(tc.tile_pool(name="rec", bufs=4))
    zt_p = ctx.enter_context(tc.tile_pool(name="zt", bufs=2))
    outs_p = ctx.enter_context(tc.tile_pool(name="outs", bufs=2))

    ps_pj = ctx.enter_context(tc.tile_pool(name="ps_pj", bufs=3, space="PSUM"))
    ps_sc = ctx.enter_context(tc.tile_pool(name="ps_sc", bufs=3, space="PSUM"))
    ps_z = ctx.enter_context(tc.tile_pool(name="ps_z", bufs=2, space="PSUM"))

    def phase_a(p):
        """Load window pair (2p, 2p+1); project q/k/v (dense PE work).
        q/k matmuls cover both windows at once (N=512)."""
        xT2 = xt_p.tile([128, 2, 4, S], BF16, tag="xT", name=f"xT{p}")
        emps = []
        for w2 in range(2):
            nc.sync.dma_start_transpose(xT2[:, w2], d["x"][2 * p + w2])
            emp = emp_p.tile([128, H, 2, S], BF16, tag="emp", name=f"emp{p}_{w2}")
            nc.sync.dma_start(emp[:], d["emp"][2 * p + w2])
            emps.append(emp)

        qT2 = qkv_p.tile([128, 2, 4, S], BF16, tag="qT", name=f"qT{p}")
        kT2 = qkv_p.tile([128, 2, 4, S], BF16, tag="kT", name=f"kT{p}")
        vAs = []
        for w2 in range(2):
            vA = qkv_p.tile([128, 2, H, 128], BF16, tag=f"vA{w2}",
                            name=f"vA{p}_{w2}")
            nc.gpsimd.tensor_copy(vA[:, :, :, 0:64], vones[:])
            vAs.append(vA)

        def qk_chunk(oc, wt, dst, bias):
            pp = ps_pj.tile([128, 2, S], F32, tag="pj", name=f"pp{p}_{wt}_{oc}")
            for ic in range(4):
                nc.tensor.matmul(pp[:], w_sb[wt][:, ic, oc * 128:(oc + 1) * 128],
                                 xT2[:, :, ic, :], start=(ic == 0), stop=(ic == 3))
            nc.scalar.activation(dst[:, :, oc, :], pp[:], AF.Identity,
                                 bias=bias[:, oc:oc + 1])

        def v_chunk(w2, sc):
            pv = ps_pj.tile([128, E], F32, tag="pj", name=f"pv{p}_{w2}_{sc}")
            for ic in range(4):
                nc.tensor.matmul(pv[:], xT2[:, w2, ic, sc * 128:(sc + 1) * 128],
                                 w_sb["wv"][:, ic], start=(ic == 0), stop=False)
            nc.tensor.matmul(pv[:], ones_b[:], bv_sb[:], start=False, stop=True)
            nc.scalar.copy(vAs[w2][:, sc, :, 64:128],
                           pv[:].rearrange("p (h o) -> p h o", h=H))

        chunks = []
        for oc in range(4):
            chunks.append(lambda oc=oc: qk_chunk(oc, "wq", qT2, bq_sb))
            chunks.append(lambda oc=oc: qk_chunk(oc, "wk", kT2, bk_sb))
        for w2 in range(2):
            chunks.append(lambda w2=w2: v_chunk(w2, 0))
            chunks.append(lambda w2=w2: v_chunk(w2, 1))
        return (qT2, kT2, vAs, emps), chunks

    def phase_b(p, qT2, kT2, vAs, emps):
        """Attention + output projection for window pair p."""
        zTs = [zt_p.tile([128, 4, S], BF16, tag=f"zT{w2}", name=f"zT{p}_{w2}")
               for w2 in range(2)]

        pending = []

        def flush():
            while pending:
                pending.pop(0)()

        def head(w2, h):
            if pending:
                pending.pop(0)()
            oc, prow = h // 2, (h % 2) * 64
            w = 2 * p + w2
            # transposed scores: [128 (j%128), jc, i]
            sT = ps_sc.tile([128, 2, S], F32, tag="sc", name=f"sT{w}_{h}")
            for jc in range(2):
                nc.tensor.matmul(sT[:, jc],
                                 kT2[prow:prow + 64, w2, oc, jc * 128:(jc + 1) * 128],
                                 qT2[prow:prow + 64, w2, oc], start=True, stop=True)
            et = et_p.tile([128, 2, S], BF16, tag="et", name=f"et{w}_{h}")
            nc.scalar.activation(et[:], sT[:], AF.Exp)
            expT = expt_p.tile([128, 2, S], BF16, tag="expT", name=f"expT{w}_{h}")
            nc.vector.tensor_tensor(expT[:], et[:], emps[w2][:, h], AOp.mult)
            # za rows 0-63 each = sum_j exp (den), rows 64-127 = v_h^T @ exp
            za = ps_z.tile([128, S], F32, tag="zz", name=f"za{w}_{h}")
            for jc in range(2):
                nc.tensor.matmul(za[:], vAs[w2][:, jc, h], expT[:, jc],
                                 start=(jc == 0), stop=(jc == 1))

            def den(za=za, w=w, w2=w2, h=h, oc=oc, prow=prow):
                rec = rec_p.tile([64, S], F32, tag="rec", name=f"rec{w}_{h}")
                nc.vector.reciprocal_approx_fast(rec[:], za[0:64, :])
                nc.vector.tensor_tensor(zTs[w2][prow:prow + 64, oc],
                                        za[64:128, :], rec[:], AOp.mult)
            pending.append(den)

        def tail(w2):
            flush()
            w = 2 * p + w2
            out_sb = outs_p.tile([128, 2, E], F32, tag="osb", name=f"osb{w}")
            for sc in range(2):
                po = ps_pj.tile([128, E], F32, tag="pj", name=f"po{w}_{sc}")
                for ec in range(4):
                    nc.tensor.matmul(po[:], zTs[w2][:, ec, sc * 128:(sc + 1) * 128],
                                     wp_sb[:, ec], start=(ec == 0), stop=False)
                nc.tensor.matmul(po[:], ones_b[:], bp_sb[:], start=False, stop=True)
                if sc == 0:
                    nc.scalar.copy(out_sb[:, sc], po[:])
                else:
                    nc.vector.tensor_copy(out_sb[:, sc], po[:])
            nc.sync.dma_start(d["out"][w].rearrange("(c p) e -> p c e", p=128), out_sb[:])

        fns = []
        for w2 in range(2):
            fns.extend(lambda h=h, w2=w2: head(w2, h) for h in range(H))
            fns.append(lambda w2=w2: tail(w2))
        return fns

    # chunks of the next pair are spread over this pair's head slots, starting
    # a few heads in so the pair's input DMAs have compute as latency cover.
    CHUNK_SLOTS = {3, 4, 5, 7, 8, 9, 11, 12, 13, 15, 16, 17}
    CHUNK_LAG = 3
    n_p = n_w // 2
    prev = None
    for p in range(n_p):
        cur, chunks = phase_a(p)
        if prev is not None:
            bfns = phase_b(p - 1, *prev)
            seq = []
            ci = 0
            for i, fn in enumerate(bfns):
                seq.append(fn)
                if i in CHUNK_SLOTS and ci < len(chunks):
                    seq.append(chunks[ci]); ci += 1
            seq.extend(chunks[ci:])
            for fn in seq:
                fn()
        else:
            for fn in chunks:
                fn()
        prev = cur
    for fn in phase_b(n_p - 1, *prev):
        fn()


def _decl(nc, n_w):
    return {
        "x": nc.dram_tensor("x", [n_w, S, E], BF16, kind="ExternalInput"),
        "emp": nc.dram_tensor("emp", [n_w, 128, H, 2, S], BF16, kind="ExternalInput"),
        "wq": nc.dram_tensor("wq", [128, 4, E], BF16, kind="ExternalInput"),
        "wk": nc.dram_tensor("wk", [128, 4, E], BF16, kind="ExternalInput"),
        "wv": nc.dram_tensor("wv", [128, 4, E], BF16, kind="ExternalInput"),
        "wp": nc.dram_tensor("wp", [128, 4, E], BF16, kind="ExternalInput"),
        "bq": nc.dram_tensor("bq", [128, 4], F32, kind="ExternalInput"),
        "bk": nc.dram_tensor("bk", [128, 4], F32, kind="ExternalInput"),
        "bv_row": nc.dram_tensor("bv_row", [1, E], BF16, kind="ExternalInput"),
        "bp_row": nc.dram_tensor("bp_row", [1, E], BF16, kind="ExternalInput"),
        "ones_b": nc.dram_tensor("ones_b", [1, 128], BF16, kind="ExternalInput"),
        "vones": nc.dram_tensor("vones", [128, 2, H, 64], BF16, kind="ExternalInput"),
        "out": nc.dram_tensor("out", [n_w, S, E], F32, kind="ExternalOutput"),
    }


def _build(n_w):
    nc = bacc.Bacc("TRN2", target_bir_lowering=False, debug=False)
    d = _decl(nc, n_w)
    from contextlib import ExitStack
    with tile.TileContext(nc) as tc, ExitStack() as ctx:
        _emit(nc, tc, ctx, n_w, d)
    nc.compile()
    return nc


_NC_CACHE = {}


def _get_nc(n_w):
    if n_w not in _NC_CACHE:
        _NC_CACHE[n_w] = _build(n_w)
    return _NC_CACHE[n_w]


def _host_prep(mask, Wq, bq, Wk, bk, Wv, bv, Wp, bp, pos_bias, n_w):
    """Shared (replicated) input tensors, host-side layout prep."""
    f = np.float32

    def chunk_w(wt, dtype):  # [out,in] torch layout -> [128 (p), 4 (ic), out]
        wt_t = np.asarray(wt, f).T  # [in, out]
        return np.ascontiguousarray(
            wt_t.reshape(4, 128, E).transpose(1, 0, 2)).astype(dtype)

    wq_t = chunk_w(np.asarray(Wq, f) * SCALE, BFNP)
    wk_t = chunk_w(Wk, BFNP)
    wv_t = chunk_w(Wv, BFNP)
    wp_t = chunk_w(Wp, BFNP)
    bq_t = np.ascontiguousarray((np.asarray(bq, f) * SCALE).reshape(4, 128).T)
    bk_t = np.ascontiguousarray(np.asarray(bk, f).reshape(4, 128).T)
    bv_row = np.asarray(bv, f).reshape(1, E).astype(BFNP)
    bp_row = np.asarray(bp, f).reshape(1, E).astype(BFNP)
    # emp = exp(mask^T + pos^T), laid out [w, p (j%128), h, jc (j//128), i]
    maskT = np.asarray(mask, f)[0, :n_w, 0].transpose(0, 2, 1)  # [w, j, i]
    posT = np.asarray(pos_bias, f).transpose(0, 2, 1)           # [h, j, i]
    empf = np.exp(maskT[:, None] + posT[None])                  # [w, h, j, i]
    emp = np.ascontiguousarray(
        empf.reshape(n_w, H, 2, 128, S).transpose(0, 3, 1, 2, 4)).astype(BFNP)
    return {
        "wq": wq_t, "wk": wk_t, "wv": wv_t, "wp": wp_t,
        "bq": bq_t, "bk": bk_t, "bv_row": bv_row, "bp_row": bp_row,
        "emp": emp,
        "ones_b": np.ones((1, 128), BFNP),
        "vones": np.ones((128, 2, H, 64), BFNP),
    }


def kernel(x, mask, Wq, bq, Wk, bk, Wv, bv, Wp, bp, pos_bias, _trace=False):
    n_w = int(os.environ.get("KERNEL_NW", W))
    n_cores = NCORES
    x = np.asarray(x, np.float32)[:, :n_w].astype(BFNP)
    shared = _host_prep(mask, Wq, bq, Wk, bk, Wv, bv, Wp, bp, pos_bias, n_w)

    in_maps = []
    for c in range(n_cores):
        m = dict(shared)
        m["x"] = np.ascontiguousarray(x[c % B])
        in_maps.append(m)

    nc = _get_nc(n_w)
    res = run_bass_kernel_spmd(nc, in_maps, list(range(n_cores)), trace=_trace,
                               tmpdir=(os.environ.get("KERNEL_TRACE_DIR") if _trace else None))
    out = np.stack([res.results[c]["out"] for c in range(B)], axis=0)
    if _trace:
        kernel._last_exec_time_ns = res.exec_time_ns
        kernel._last_results = res
    return out


# revision 15
# speedup vs baseline: 1.1627x; 1.1271x over previous
"""Trainium2 Bass kernel for batched windowed multi-head attention.

Shapes: x (8, 64, 256, 512) f32, H=8 heads, D=64.
Sharding: data-parallel over batch dim B=8 -> 1 batch row per NeuronCore.

v2 design (vs baseline):
- x cast to bf16 on host; loaded pre-transposed via the XBAR DMA-transpose
  (no PE transposes, no PSUM->SBUF copies for xT).
- q/k/v projections in bf16 (same PE rate as fp32r at N>=256, half the
  SBUF/DMA traffic); q/k bias folded into the PSUM->SBUF cast on ACT.
- mask+pos_bias folded multiplicatively: host precomputes
  emp = exp(mask^T + pos^T) in bf16, streamed per window over DMA; on-chip
  softmax numerator is exp(scores) * emp via one DVE bf16 multiply per head
  (replaces the Pool add + DVE add chains of the baseline).
- denominators via a 64-wide ones BLOCK appended to V: za = [v|1]^T @ exp
  gives rows 64..127 all equal to the softmax denominator, so the
  reciprocal + normalize are two plain DVE ops, no broadcasts needed.
- v/out biases folded into the projection matmuls via a K=1 ones-row
  matmul (PE) instead of DVE scalar_tensor_tensor ops.
"""
import os
import numpy as np
import ml_dtypes

import concourse.bass as bass
import concourse.mybir as mybir
import concourse.tile as tile
from concourse import bacc
from concourse.bass_utils import run_bass_kernel_spmd

B, W, S, E = 8, 64, 256, 512
H, D = 8, 64
SCALE = D ** -0.5
NCORES = 8
F32 = mybir.dt.float32
F32R = mybir.dt.float32r
BF16 = mybir.dt.bfloat16
AOp = mybir.AluOpType
AF = mybir.ActivationFunctionType
BFNP = ml_dtypes.bfloat16


def _emit(nc, tc, ctx, n_w, d):
    """Emit the per-core program: n_w windows of MHA."""
    const = ctx.enter_context(tc.tile_pool(name="const", bufs=1))

    # --- one-time: weights, biases ---
    w_sb = {}
    for name in ("wq", "wk", "wv"):
        t = const.tile([128, 4, E], BF16, tag=name)
        nc.sync.dma_start(t[:], d[name][:])
        w_sb[name] = t
    wp_sb = const.tile([128, 4, E], BF16, tag="wp")
    nc.sync.dma_start(wp_sb[:], d["wp"][:])
    bp_sb = const.tile([1, E], BF16, tag="bp")
    nc.sync.dma_start(bp_sb[:], d["bp_row"][:])

    bq_sb = const.tile([128, 4], F32)
    nc.sync.dma_start(bq_sb[:], d["bq"][:])
    bk_sb = const.tile([128, 4], F32)
    nc.sync.dma_start(bk_sb[:], d["bk"][:])
    bv_sb = const.tile([1, E], BF16)
    nc.sync.dma_start(bv_sb[:], d["bv_row"][:])

    ones_b = const.tile([1, 128], BF16)
    nc.sync.dma_start(ones_b[:], d["ones_b"][:])
    vones = const.tile([128, 2, H, 64], BF16)
    nc.sync.dma_start(vones[:], d["vones"][:])

    # --- pools for the per-window-pair pipeline ---
    emp_p = ctx.enter_context(tc.tile_pool(name="emp", bufs=4))
    xt_p = ctx.enter_context(tc.tile_pool(name="xt", bufs=2))
    qkv_p = ctx.enter_context(tc.tile_pool(name="qkv", bufs=2))
    et_p = ctx.enter_context(tc.tile_pool(name="et", bufs=3))
    expt_p = ctx.enter_context(tc.tile_pool(name="expt", bufs=4))
    rec_p = ctx.enter_context(tc.tile_pool(name="rec", bufs=4))
    zt_p = ctx.enter_context(tc.tile_pool(name="zt", bufs=2))
    outs_p = ctx.enter_context(tc.tile_pool(name="outs", bufs=2))

    ps_pj = ctx.enter_context(tc.tile_pool(name="ps_pj", bufs=3, space="PSUM"))
    ps_sc = ctx.enter_context(tc.tile_pool(name="ps_sc", bufs=3, space="PSUM"))
    ps_z = ctx.enter_context(tc.tile_pool(name="ps_z", bufs=2, space="PSUM"))

    def phase_a(p):
        """Load window pair (2p, 2p+1); project q/k/v (dense PE work).
        q/k matmuls cover both windows at once (N=512)."""
        xT2 = xt_p.tile([128, 2, 4, S], BF16, tag="xT", name=f"xT{p}")
        emps = []
        for w2 in range(2):
            nc.sync.dma_start_transpose(xT2[:, w2], d["x"][2 * p + w2])
            emp = emp_p.tile([128, H, 2, S], BF16, tag="emp", name=f"emp{p}_{w2}")
            nc.sync.dma_start(emp[:], d["emp"][2 * p + w2])
            emps.append(emp)

        qT2 = qkv_p.tile([128, 2, 4, S], BF16, tag="qT", name=f"qT{p}")
        kT2 = qkv_p.tile([128, 2, 4, S], BF16, tag="kT", name=f"kT{p}")
        # vA[..., 0:64] = ones (den), [..., 64:128] = v_h. The ones copy
        # runs on ACT: GpSimd's slow Q7 copy stalls DVE/PE via SBUF ports.
        vAs = []
        for w2 in range(2):
            vA = qkv_p.tile([128, 2, H, 128], BF16, tag=f"vA{w2}",
                            name=f"vA{p}_{w2}")
            nc.scalar.copy(vA[:, :, :, 0:64], vones[:])
            vAs.append(vA)

        def qk_chunk(oc, wt, dst, bias):
            pp = ps_pj.tile([128, 2, S], F32, tag="pj", name=f"pp{p}_{wt}_{oc}")
            for ic in range(4):
                nc.tensor.matmul(pp[:], w_sb[wt][:, ic, oc * 128:(oc + 1) * 128],
                                 xT2[:, :, ic, :], start=(ic == 0), stop=(ic == 3))
            nc.scalar.activation(dst[:, :, oc, :], pp[:], AF.Identity,
                                 bias=bias[:, oc:oc + 1])

        def v_chunk(w2, sc):
            pv = ps_pj.tile([128, E], F32, tag="pj", name=f"pv{p}_{w2}_{sc}")
            for ic in range(4):
                nc.tensor.matmul(pv[:], xT2[:, w2, ic, sc * 128:(sc + 1) * 128],
                                 w_sb["wv"][:, ic], start=(ic == 0), stop=False)
            nc.tensor.matmul(pv[:], ones_b[:], bv_sb[:], start=False, stop=True)
            nc.scalar.copy(vAs[w2][:, sc, :, 64:128],
                           pv[:].rearrange("p (h o) -> p h o", h=H))

        chunks = []
        for oc in range(4):
            chunks.append(lambda oc=oc: qk_chunk(oc, "wq", qT2, bq_sb))
            chunks.append(lambda oc=oc: qk_chunk(oc, "wk", kT2, bk_sb))
        for w2 in range(2):
            chunks.append(lambda w2=w2: v_chunk(w2, 0))
            chunks.append(lambda w2=w2: v_chunk(w2, 1))
        return (qT2, kT2, vAs, emps), chunks

    def phase_b(p, qT2, kT2, vAs, emps):
        """Attention + output projection for window pair p."""
        zTs = [zt_p.tile([128, 4, S], BF16, tag=f"zT{w2}", name=f"zT{p}_{w2}")
               for w2 in range(2)]

        pending = []

        def flush():
            while pending:
                pending.pop(0)()

        def head(w2, h):
            if pending:
                pending.pop(0)()
            oc, prow = h // 2, (h % 2) * 64
            w = 2 * p + w2
            # transposed scores: [128 (j%128), jc, i]
            sT = ps_sc.tile([128, 2, S], F32, tag="sc", name=f"sT{w}_{h}")
            for jc in range(2):
                nc.tensor.matmul(sT[:, jc],
                                 kT2[prow:prow + 64, w2, oc, jc * 128:(jc + 1) * 128],
                                 qT2[prow:prow + 64, w2, oc], start=True, stop=True)
            et = et_p.tile([128, 2, S], BF16, tag="et", name=f"et{w}_{h}")
            nc.scalar.activation(et[:], sT[:], AF.Exp)
            expT = expt_p.tile([128, 2, S], BF16, tag="expT", name=f"expT{w}_{h}")
            nc.vector.tensor_tensor(expT[:], et[:], emps[w2][:, h], AOp.mult)
            # za rows 0-63 each = sum_j exp (den), rows 64-127 = v_h^T @ exp
            za = ps_z.tile([128, S], F32, tag="zz", name=f"za{w}_{h}")
            for jc in range(2):
                nc.tensor.matmul(za[:], vAs[w2][:, jc, h],
                                 expT[:, jc], start=(jc == 0), stop=(jc == 1))

            def den(za=za, w=w, w2=w2, h=h, oc=oc, prow=prow):
                rec = rec_p.tile([64, S], F32, tag="rec", name=f"rec{w}_{h}")
                nc.vector.reciprocal_approx_fast(rec[:], za[0:64, :])
                nc.vector.tensor_tensor(zTs[w2][prow:prow + 64, oc],
                                        za[64:128, :], rec[:], AOp.mult)
            pending.append(den)

        def tail(w2):
            flush()
            w = 2 * p + w2
            out_sb = outs_p.tile([128, 2, E], F32, tag="osb", name=f"osb{w}")
            for sc in range(2):
                po = ps_pj.tile([128, E], F32, tag="pj", name=f"po{w}_{sc}")
                for ec in range(4):
                    nc.tensor.matmul(po[:], zTs[w2][:, ec, sc * 128:(sc + 1) * 128],
                                     wp_sb[:, ec], start=(ec == 0), stop=False)
                nc.tensor.matmul(po[:], ones_b[:], bp_sb[:], start=False, stop=True)
                if sc == 0:
                    nc.scalar.copy(out_sb[:, sc], po[:])
                else:
                    nc.vector.tensor_copy(out_sb[:, sc], po[:])
            nc.sync.dma_start(d["out"][w].rearrange("(c p) e -> p c e", p=128), out_sb[:])

        fns = []
        for w2 in range(2):
            fns.extend(lambda h=h, w2=w2: head(w2, h) for h in range(H))
            fns.append(lambda w2=w2: tail(w2))
        return fns

    # chunks of the next pair are spread over this pair's head slots, starting
    # a few heads in so the pair's input DMAs have compute as latency cover.
    CHUNK_SLOTS = {3, 4, 5, 7, 8, 9, 11, 12, 13, 15, 16, 17}
    CHUNK_LAG = 3
    n_p = n_w // 2
    prev = None
    for p in range(n_p):
        cur, chunks = phase_a(p)
        if prev is not None:
            bfns = phase_b(p - 1, *prev)
            seq = []
            ci = 0
            for i, fn in enumerate(bfns):
                seq.append(fn)
                if i in CHUNK_SLOTS and ci < len(chunks):
                    seq.append(chunks[ci]); ci += 1
            seq.extend(chunks[ci:])
            for fn in seq:
                fn()
        else:
            for fn in chunks:
                fn()
        prev = cur
    for fn in phase_b(n_p - 1, *prev):
        fn()


def _decl(nc, n_w):
    return {
        "x": nc.dram_tensor("x", [n_w, S, E], BF16, kind="ExternalInput"),
        "emp": nc.dram_tensor("emp", [n_w, 128, H, 2, S], BF16, kind="ExternalInput"),
        "wq": nc.dram_tensor("wq", [128, 4, E], BF16, kind="ExternalInput"),
        "wk": nc.dram_tensor("wk", [128, 4, E], BF16, kind="ExternalInput"),
        "wv": nc.dram_tensor("wv", [128, 4, E], BF16, kind="ExternalInput"),
        "wp": nc.dram_tensor("wp", [128, 4, E], BF16, kind="ExternalInput"),
        "bq": nc.dram_tensor("bq", [128, 4], F32, kind="ExternalInput"),
        "bk": nc.dram_tensor("bk", [128, 4], F32, kind="ExternalInput"),
        "bv_row": nc.dram_tensor("bv_row", [1, E], BF16, kind="ExternalInput"),
        "bp_row": nc.dram_tensor("bp_row", [1, E], BF16, kind="ExternalInput"),
        "ones_b": nc.dram_tensor("ones_b", [1, 128], BF16, kind="ExternalInput"),
        "vones": nc.dram_tensor("vones", [128, 2, H, 64], BF16, kind="ExternalInput"),
        "out": nc.dram_tensor("out", [n_w, S, E], F32, kind="ExternalOutput"),
    }


def _build(n_w):
    nc = bacc.Bacc("TRN2", target_bir_lowering=False, debug=False)
    d = _decl(nc, n_w)
    from contextlib import ExitStack
    with tile.TileContext(nc) as tc, ExitStack() as ctx:
        _emit(nc, tc, ctx, n_w, d)
    nc.compile()
    return nc


_NC_CACHE = {}


def _get_nc(n_w):
    if n_w not in _NC_CACHE:
        _NC_CACHE[n_w] = _build(n_w)
    return _NC_CACHE[n_w]


def _host_prep(mask, Wq, bq, Wk, bk, Wv, bv, Wp, bp, pos_bias, n_w):
    """Shared (replicated) input tensors, host-side layout prep."""
    f = np.float32

    def chunk_w(wt, dtype):  # [out,in] torch layout -> [128 (p), 4 (ic), out]
        wt_t = np.asarray(wt, f).T  # [in, out]
        return np.ascontiguousarray(
            wt_t.reshape(4, 128, E).transpose(1, 0, 2)).astype(dtype)

    wq_t = chunk_w(np.asarray(Wq, f) * SCALE, BFNP)
    wk_t = chunk_w(Wk, BFNP)
    wv_t = chunk_w(Wv, BFNP)
    wp_t = chunk_w(Wp, BFNP)
    bq_t = np.ascontiguousarray((np.asarray(bq, f) * SCALE).reshape(4, 128).T)
    bk_t = np.ascontiguousarray(np.asarray(bk, f).reshape(4, 128).T)
    bv_row = np.asarray(bv, f).reshape(1, E).astype(BFNP)
    bp_row = np.asarray(bp, f).reshape(1, E).astype(BFNP)
    # emp = exp(mask^T + pos^T), laid out [w, p (j%128), h, jc (j//128), i]
    maskT = np.asarray(mask, f)[0, :n_w, 0].transpose(0, 2, 1)  # [w, j, i]
    posT = np.asarray(pos_bias, f).transpose(0, 2, 1)           # [h, j, i]
    empf = np.exp(maskT[:, None] + posT[None])                  # [w, h, j, i]
    emp = np.ascontiguousarray(
        empf.reshape(n_w, H, 2, 128, S).transpose(0, 3, 1, 2, 4)).astype(BFNP)
    return {
        "wq": wq_t, "wk": wk_t, "wv": wv_t, "wp": wp_t,
        "bq": bq_t, "bk": bk_t, "bv_row": bv_row, "bp_row": bp_row,
        "emp": emp,
        "ones_b": np.ones((1, 128), BFNP),
        "vones": np.ones((128, 2, H, 64), BFNP),
    }


def kernel(x, mask, Wq, bq, Wk, bk, Wv, bv, Wp, bp, pos_bias, _trace=False):
    n_w = int(os.environ.get("KERNEL_NW", W))
    n_cores = NCORES
    x = np.asarray(x, np.float32)[:, :n_w].astype(BFNP)
    shared = _host_prep(mask, Wq, bq, Wk, bk, Wv, bv, Wp, bp, pos_bias, n_w)

    in_maps = []
    for c in range(n_cores):
        m = dict(shared)
        m["x"] = np.ascontiguousarray(x[c % B])
        in_maps.append(m)

    nc = _get_nc(n_w)
    res = run_bass_kernel_spmd(nc, in_maps, list(range(n_cores)), trace=_trace,
                               tmpdir=(os.environ.get("KERNEL_TRACE_DIR") if _trace else None))
    out = np.stack([res.results[c]["out"] for c in range(B)], axis=0)
    if _trace:
        kernel._last_exec_time_ns = res.exec_time_ns
        kernel._last_results = res
    return out


# revision 18
# speedup vs baseline: 1.2629x; 1.0862x over previous
"""Trainium2 Bass kernel for batched windowed multi-head attention.

Shapes: x (8, 64, 256, 512) f32, H=8 heads, D=64.
Sharding: data-parallel over batch dim B=8 -> 1 batch row per NeuronCore.

v2 design (vs baseline):
- x cast to bf16 on host; loaded pre-transposed via the XBAR DMA-transpose
  (no PE transposes, no PSUM->SBUF copies for xT).
- q/k/v projections in bf16 (same PE rate as fp32r at N>=256, half the
  SBUF/DMA traffic); q/k bias folded into the PSUM->SBUF cast on ACT.
- mask+pos_bias folded multiplicatively: host precomputes
  emp = exp(mask^T + pos^T) in bf16, streamed per window over DMA; on-chip
  softmax numerator is exp(scores) * emp via one DVE bf16 multiply per head
  (replaces the Pool add + DVE add chains of the baseline).
- denominators via a 64-wide ones BLOCK appended to V: za = [v|1]^T @ exp
  gives rows 64..127 all equal to the softmax denominator, so the
  reciprocal + normalize are two plain DVE ops, no broadcasts needed.
- v/out biases folded into the projection matmuls via a K=1 ones-row
  matmul (PE) instead of DVE scalar_tensor_tensor ops.
"""
import os
import numpy as np
import ml_dtypes

import concourse.bass as bass
import concourse.mybir as mybir
import concourse.tile as tile
from concourse import bacc
from concourse.bass_utils import run_bass_kernel_spmd

B, W, S, E = 8, 64, 256, 512
H, D = 8, 64
SCALE = D ** -0.5
NCORES = 8
F32 = mybir.dt.float32
F32R = mybir.dt.float32r
BF16 = mybir.dt.bfloat16
AOp = mybir.AluOpType
AF = mybir.ActivationFunctionType
BFNP = ml_dtypes.bfloat16


def _emit(nc, tc, ctx, n_w, d):
    """Emit the per-core program: n_w windows of MHA."""
    const = ctx.enter_context(tc.tile_pool(name="const", bufs=1))

    # --- one-time: weights, biases. q/k weights load first; v/p weights and
    # the output bias are deferred into the first pair's chunk stream so
    # window-0 projections start as early as possible. ---
    w_sb = {}
    for name in ("wq", "wk"):
        t = const.tile([128, 4, E], BF16, tag=name)
        nc.sync.dma_start(t[:], d[name][:])
        w_sb[name] = t
    w_sb["wv"] = const.tile([128, 4, E], BF16, tag="wv", name="wv_sb")
    wp_sb = const.tile([128, 4, E], BF16, tag="wp")
    bp_bc = const.tile([128, E], F32, tag="bp")

    def late_consts():
        nc.sync.dma_start(w_sb["wv"][:], d["wv"][:])
        nc.sync.dma_start(wp_sb[:], d["wp"][:])
        nc.sync.dma_start(bp_bc[:], d["bp_bc"][:])

    bq_sb = const.tile([128, 4], F32)
    nc.sync.dma_start(bq_sb[:], d["bq"][:])
    bk_sb = const.tile([128, 4], F32)
    nc.sync.dma_start(bk_sb[:], d["bk"][:])
    bv_sb = const.tile([1, E], BF16)
    nc.sync.dma_start(bv_sb[:], d["bv_row"][:])

    ones_b = const.tile([1, 128], BF16)
    nc.sync.dma_start(ones_b[:], d["ones_b"][:])

    # --- pools for the per-window-pair pipeline ---
    emp_p = ctx.enter_context(tc.tile_pool(name="emp", bufs=4))
    xt_p = ctx.enter_context(tc.tile_pool(name="xt", bufs=2))
    qkv_p = ctx.enter_context(tc.tile_pool(name="qkv", bufs=2))
    et_p = ctx.enter_context(tc.tile_pool(name="et", bufs=3))
    expt_p = ctx.enter_context(tc.tile_pool(name="expt", bufs=4))
    rec_p = ctx.enter_context(tc.tile_pool(name="rec", bufs=4))
    zt_p = ctx.enter_context(tc.tile_pool(name="zt", bufs=2))
    outs_p = ctx.enter_context(tc.tile_pool(name="outs", bufs=2))

    ps_pj = ctx.enter_context(tc.tile_pool(name="ps_pj", bufs=3, space="PSUM"))
    ps_sc = ctx.enter_context(tc.tile_pool(name="ps_sc", bufs=3, space="PSUM"))
    ps_z = ctx.enter_context(tc.tile_pool(name="ps_z", bufs=2, space="PSUM"))

    def phase_a(p):
        """Load window pair (2p, 2p+1); project q/k/v (dense PE work).
        q/k matmuls cover both windows at once (N=512)."""
        xT2 = xt_p.tile([128, 2, 4, S], BF16, tag="xT", name=f"xT{p}")
        emps = []
        for w2 in range(2):
            nc.sync.dma_start_transpose(xT2[:, w2], d["x"][2 * p + w2])
            emp = emp_p.tile([128, H, 2, S], BF16, tag="emp", name=f"emp{p}_{w2}")
            nc.sync.dma_start(emp[:], d["emp"][2 * p + w2])
            emps.append(emp)

        qT2 = qkv_p.tile([128, 2, 4, S], BF16, tag="qT", name=f"qT{p}")
        kT2 = qkv_p.tile([128, 2, 4, S], BF16, tag="kT", name=f"kT{p}")
        # vA[..., 0:64] = ones (den), [..., 64:128] = v_h. The ones copy
        # runs on ACT: GpSimd's slow Q7 copy stalls DVE/PE via SBUF ports.
        vAs = []
        for w2 in range(2):
            vA = qkv_p.tile([128, 2, H, 128], BF16, tag=f"vA{w2}",
                            name=f"vA{p}_{w2}")
            nc.sync.dma_start(vA[:, :, :, 0:64], d["vones"][:])
            vAs.append(vA)

        def qk_chunk(oc, wt, dst, bias):
            pp = ps_pj.tile([128, 2, S], F32, tag="pj", name=f"pp{p}_{wt}_{oc}")
            for ic in range(4):
                nc.tensor.matmul(pp[:], w_sb[wt][:, ic, oc * 128:(oc + 1) * 128],
                                 xT2[:, :, ic, :], start=(ic == 0), stop=(ic == 3))
            nc.scalar.activation(dst[:, :, oc, :], pp[:], AF.Identity,
                                 bias=bias[:, oc:oc + 1])

        def v_chunk(w2, sc):
            pv = ps_pj.tile([128, E], F32, tag="pj", name=f"pv{p}_{w2}_{sc}")
            for ic in range(4):
                nc.tensor.matmul(pv[:], xT2[:, w2, ic, sc * 128:(sc + 1) * 128],
                                 w_sb["wv"][:, ic], start=(ic == 0), stop=False)
            nc.tensor.matmul(pv[:], ones_b[:], bv_sb[:], start=False, stop=True)
            nc.scalar.copy(vAs[w2][:, sc, :, 64:128],
                           pv[:].rearrange("p (h o) -> p h o", h=H))

        chunks = []
        for oc in range(4):
            chunks.append(lambda oc=oc: qk_chunk(oc, "wq", qT2, bq_sb))
            chunks.append(lambda oc=oc: qk_chunk(oc, "wk", kT2, bk_sb))
        for w2 in range(2):
            chunks.append(lambda w2=w2: v_chunk(w2, 0))
            chunks.append(lambda w2=w2: v_chunk(w2, 1))
        return (qT2, kT2, vAs, emps), chunks

    def phase_b(p, qT2, kT2, vAs, emps):
        """Attention + output projection for window pair p."""
        zTs = [zt_p.tile([128, 4, S], BF16, tag=f"zT{w2}", name=f"zT{p}_{w2}")
               for w2 in range(2)]

        pending = []

        def flush():
            while pending:
                pending.pop(0)()

        def head(w2, h):
            if pending:
                pending.pop(0)()
            oc, prow = h // 2, (h % 2) * 64
            w = 2 * p + w2
            # transposed scores: [128 (j%128), jc, i]
            sT = ps_sc.tile([128, 2, S], F32, tag="sc", name=f"sT{w}_{h}")
            for jc in range(2):
                nc.tensor.matmul(sT[:, jc],
                                 kT2[prow:prow + 64, w2, oc, jc * 128:(jc + 1) * 128],
                                 qT2[prow:prow + 64, w2, oc], start=True, stop=True)
            et = et_p.tile([128, 2, S], BF16, tag="et", name=f"et{w}_{h}")
            nc.scalar.activation(et[:], sT[:], AF.Exp)
            expT = expt_p.tile([128, 2, S], BF16, tag="expT", name=f"expT{w}_{h}")
            nc.vector.tensor_tensor(expT[:], et[:], emps[w2][:, h], AOp.mult)
            # za rows 0-63 each = sum_j exp (den), rows 64-127 = v_h^T @ exp
            za = ps_z.tile([128, S], F32, tag="zz", name=f"za{w}_{h}")
            for jc in range(2):
                nc.tensor.matmul(za[:], vAs[w2][:, jc, h],
                                 expT[:, jc], start=(jc == 0), stop=(jc == 1))

            def den(za=za, w=w, w2=w2, h=h, oc=oc, prow=prow):
                rec = rec_p.tile([64, S], F32, tag="rec", name=f"rec{w}_{h}")
                nc.vector.reciprocal_approx_fast(rec[:], za[0:64, :])
                nc.vector.tensor_tensor(zTs[w2][prow:prow + 64, oc],
                                        za[64:128, :], rec[:], AOp.mult)
            pending.append(den)

        def tail(w2):
            flush()
            w = 2 * p + w2
            out_sb = outs_p.tile([128, 2, E], F32, tag="osb", name=f"osb{w}")
            for sc in range(2):
                po = ps_pj.tile([128, E], F32, tag="pj", name=f"po{w}_{sc}")
                for ec in range(4):
                    nc.tensor.matmul(po[:], zTs[w2][:, ec, sc * 128:(sc + 1) * 128],
                                     wp_sb[:, ec], start=(ec == 0), stop=(ec == 3))
                nc.vector.tensor_tensor(out_sb[:, sc], po[:], bp_bc[:], AOp.add)
            nc.sync.dma_start(d["out"][w].rearrange("(c p) e -> p c e", p=128), out_sb[:])

        fns = []
        for w2 in range(2):
            fns.extend(lambda h=h, w2=w2: head(w2, h) for h in range(H))
            fns.append(lambda w2=w2: tail(w2))
        return fns

    # chunks of the next pair are spread over this pair's head slots, starting
    # a few heads in so the pair's input DMAs have compute as latency cover.
    CHUNK_SLOTS = {3, 4, 5, 7, 8, 9, 11, 12, 13, 15, 16, 17}
    CHUNK_LAG = 3
    n_p = n_w // 2
    prev = None
    for p in range(n_p):
        cur, chunks = phase_a(p)
        if prev is not None:
            bfns = phase_b(p - 1, *prev)
            seq = []
            ci = 0
            for i, fn in enumerate(bfns):
                seq.append(fn)
                if i in CHUNK_SLOTS and ci < len(chunks):
                    seq.append(chunks[ci]); ci += 1
            seq.extend(chunks[ci:])
            for fn in seq:
                fn()
        else:
            for i, fn in enumerate(chunks):
                fn()
                if i == 1:
                    late_consts()
        prev = cur
    for fn in phase_b(n_p - 1, *prev):
        fn()


def _decl(nc, n_w):
    return {
        "x": nc.dram_tensor("x", [n_w, S, E], BF16, kind="ExternalInput"),
        "emp": nc.dram_tensor("emp", [n_w, 128, H, 2, S], BF16, kind="ExternalInput"),
        "wq": nc.dram_tensor("wq", [128, 4, E], BF16, kind="ExternalInput"),
        "wk": nc.dram_tensor("wk", [128, 4, E], BF16, kind="ExternalInput"),
        "wv": nc.dram_tensor("wv", [128, 4, E], BF16, kind="ExternalInput"),
        "wp": nc.dram_tensor("wp", [128, 4, E], BF16, kind="ExternalInput"),
        "bq": nc.dram_tensor("bq", [128, 4], F32, kind="ExternalInput"),
        "bk": nc.dram_tensor("bk", [128, 4], F32, kind="ExternalInput"),
        "bv_row": nc.dram_tensor("bv_row", [1, E], BF16, kind="ExternalInput"),
        "bp_bc": nc.dram_tensor("bp_bc", [128, E], F32, kind="ExternalInput"),
        "ones_b": nc.dram_tensor("ones_b", [1, 128], BF16, kind="ExternalInput"),
        "vones": nc.dram_tensor("vones", [128, 2, H, 64], BF16, kind="ExternalInput"),
        "out": nc.dram_tensor("out", [n_w, S, E], F32, kind="ExternalOutput"),
    }


def _build(n_w):
    nc = bacc.Bacc("TRN2", target_bir_lowering=False, debug=False)
    d = _decl(nc, n_w)
    from contextlib import ExitStack
    with tile.TileContext(nc) as tc, ExitStack() as ctx:
        _emit(nc, tc, ctx, n_w, d)
    nc.compile()
    return nc


_NC_CACHE = {}


def _get_nc(n_w):
    if n_w not in _NC_CACHE:
        _NC_CACHE[n_w] = _build(n_w)
    return _NC_CACHE[n_w]


def _host_prep(mask, Wq, bq, Wk, bk, Wv, bv, Wp, bp, pos_bias, n_w):
    """Shared (replicated) input tensors, host-side layout prep."""
    f = np.float32

    def chunk_w(wt, dtype):  # [out,in] torch layout -> [128 (p), 4 (ic), out]
        wt_t = np.asarray(wt, f).T  # [in, out]
        return np.ascontiguousarray(
            wt_t.reshape(4, 128, E).transpose(1, 0, 2)).astype(dtype)

    wq_t = chunk_w(np.asarray(Wq, f) * SCALE, BFNP)
    wk_t = chunk_w(Wk, BFNP)
    wv_t = chunk_w(Wv, BFNP)
    wp_t = chunk_w(Wp, BFNP)
    bq_t = np.ascontiguousarray((np.asarray(bq, f) * SCALE).reshape(4, 128).T)
    bk_t = np.ascontiguousarray(np.asarray(bk, f).reshape(4, 128).T)
    bv_row = np.asarray(bv, f).reshape(1, E).astype(BFNP)
    bp_bc = np.ascontiguousarray(
        np.broadcast_to(np.asarray(bp, f)[None, :], (128, E)))
    # emp = exp(mask^T + pos^T), laid out [w, p (j%128), h, jc (j//128), i]
    maskT = np.asarray(mask, f)[0, :n_w, 0].transpose(0, 2, 1)  # [w, j, i]
    posT = np.asarray(pos_bias, f).transpose(0, 2, 1)           # [h, j, i]
    empf = np.exp(maskT[:, None] + posT[None])                  # [w, h, j, i]
    emp = np.ascontiguousarray(
        empf.reshape(n_w, H, 2, 128, S).transpose(0, 3, 1, 2, 4)).astype(BFNP)
    return {
        "wq": wq_t, "wk": wk_t, "wv": wv_t, "wp": wp_t,
        "bq": bq_t, "bk": bk_t, "bv_row": bv_row, "bp_bc": bp_bc,
        "emp": emp,
        "ones_b": np.ones((1, 128), BFNP),
        "vones": np.ones((128, 2, H, 64), BFNP),
    }


def kernel(x, mask, Wq, bq, Wk, bk, Wv, bv, Wp, bp, pos_bias, _trace=False):
    n_w = int(os.environ.get("KERNEL_NW", W))
    n_cores = NCORES
    x = np.asarray(x, np.float32)[:, :n_w].astype(BFNP)
    shared = _host_prep(mask, Wq, bq, Wk, bk, Wv, bv, Wp, bp, pos_bias, n_w)

    in_maps = []
    for c in range(n_cores):
        m = dict(shared)
        m["x"] = np.ascontiguousarray(x[c % B])
        in_maps.append(m)

    nc = _get_nc(n_w)
    res = run_bass_kernel_spmd(nc, in_maps, list(range(n_cores)), trace=_trace,
                               tmpdir=(os.environ.get("KERNEL_TRACE_DIR") if _trace else None))
    out = np.stack([res.results[c]["out"] for c in range(B)], axis=0)
    if _trace:
        kernel._last_exec_time_ns = res.exec_time_ns
        kernel._last_results = res
    return out
